# revision 7
# baseline (speedup 1.0000x reference)
"""Trainium2 Bass kernel for nn_AttentionFlow (additive attention + Keras GRU).

Data-parallel over 8 NeuronCores: B*T = 16 independent utterances, 2 per core.
Per utterance (x in [S=128, D=256]):
  left  = x @ w1 ; right = x @ w2
  logits[m,n] = sum_d v3[d] * tanh(left[m,d] + right[n,d] + bias[d])
  score = softmax(logits, axis=n) ; c = score @ x
  p = KerasGRU([x, c]) with h0 = 0  (reset_after=True)

Device layout choices (per core, 2 utterances):
  - SUM build on DVE in [d_half_p, (m, n)] layout, bf16 2x mode via a
    duplicated-left trick (left stored twice along free so innermost stride=1).
  - tanh on ACT with large free dims.
  - logits on PE: stationary = contiguous tanh tile [d_half, 128] per m (FWL),
    moving = v3 half column -> PSUM logitsT [n, m].
  - softmax normalization folded into c: c_un = E^T-matmul, then per-partition
    (per-m) scale by 1/rowsum.
  - GRU: R chunks stationary [128,128] (12 per step), moving = h [128, 2],
    output rp^T in [3H-chunk partitions, (chunk,u)] so gate math runs on
    [128, few]-tiles; h kept as [128, (k, u)] bf16.
"""

import functools
import sys

import numpy as np

sys.path.insert(0, "/opt/trn_rl_repo")

import ml_dtypes  # noqa: E402

import bass_rust  # noqa: E402
import concourse.bass as bass  # noqa: E402
import concourse.tile as tile  # noqa: E402
from concourse import mybir  # noqa: E402
from concourse.bass_utils import run_bass_kernel_spmd  # noqa: E402
from concourse.tile_scheduler import N_PROCS  # noqa: E402
from concourse.vector_clock import ScopedClock, VectorClock  # noqa: E402


def _patched_drain_and_barrier(self, tick_clock, wait_clock):
    # This walrus build rejects instructions carrying many sem waits
    # ("Too many sync wait commands"); spread the kernel-tail drain's waits
    # over one drain instruction per logical processor.
    g = tick_clock.global_clock
    for p in range(N_PROCS):
        try:
            v = g[p]
        except Exception:
            v = 0
        if v <= 0:
            continue
        onehot = VectorClock([g[q] if q == p else 0 for q in range(N_PROCS)])
        di = self.nc.sync.drain()
        wait_clock.add_sem_waits(di.ins, ScopedClock({None: onehot}))
    self.nc.all_engine_barrier()
    popped = self.nc._tile_sem_poison_stack.pop()
    assert popped is self._sem_poison
    self.nc.clear_and_free_semaphores(list(self.sems.allocated().values()))
    self.nc.all_engine_barrier()


tile.TileContext._drain_and_barrier = _patched_drain_and_barrier

_wsplit_counter = [0]
_orig_add_instruction = tile.TileContext._add_instruction


def _patched_add_instruction(self, inst):
    # Split multi-wait instructions: this walrus accepts at most one sync
    # wait per instruction, so carry extras on preceding EventSemaphore nops.
    si = inst.sync_info
    waits = list(si.on_wait) if si and si.on_wait else []
    if len(waits) > 1:
        for w in waits[:-1]:
            _wsplit_counter[0] += 1
            c = mybir.InstEventSemaphore.__new__(mybir.InstEventSemaphore)
            c.name = f"wsplit_{_wsplit_counter[0]}"
            c.engine = inst.engine
            c.sync_info = bass_rust.SyncInfo(on_wait=[w], on_update=[])
            _orig_add_instruction(self, c)
        inst.sync_info = bass_rust.SyncInfo(
            on_wait=[waits[-1]], on_update=list(si.on_update or [])
        )
    _orig_add_instruction(self, inst)


tile.TileContext._add_instruction = _patched_add_instruction

BF = mybir.dt.bfloat16
F32 = mybir.dt.float32
AF = mybir.ActivationFunctionType
ALU = mybir.AluOpType
BF_NP = ml_dtypes.bfloat16

B, T, S, D = 4, 4, 128, 256
H = D
NCORES = 8
U = 2  # utterances per core
MCH = 32  # m-chunk size for the SUM/tanh/logits pipeline
NCHUNK = S // MCH


def _build_graph():
    nc = bass.Bass("TRN2", target_bir_lowering=False, debug=False)

    # ---- DRAM parameters (host-preprocessed layouts) ----
    d_xbf = nc.dram_tensor("xbf", [U, S, D], BF, kind="ExternalInput").ap()
    d_xt = nc.dram_tensor("xtbf", [U, D, S], BF, kind="ExternalInput").ap()
    d_w1 = nc.dram_tensor("w1bf", [D, D], BF, kind="ExternalInput").ap()
    d_w2 = nc.dram_tensor("w2bf", [D, D], BF, kind="ExternalInput").ap()
    d_biasv = nc.dram_tensor("biasv", [128, 2], F32, kind="ExternalInput").ap()
    d_v3 = nc.dram_tensor("v3bf", [128, 2], BF, kind="ExternalInput").ap()
    d_gk = nc.dram_tensor("gkbf", [2 * D, 3 * H], BF, kind="ExternalInput").ap()
    d_rk = nc.dram_tensor("rkbf", [H, 3 * H], BF, kind="ExternalInput").ap()
    d_btot = nc.dram_tensor("btot", [128, 8], F32, kind="ExternalInput").ap()
    d_ones = nc.dram_tensor("onesbf", [128, 1], BF, kind="ExternalInput").ap()
    d_idbf = nc.dram_tensor("idbf", [128, 128], BF, kind="ExternalInput").ap()
    d_idf = nc.dram_tensor("idf32", [128, 128], F32, kind="ExternalInput").ap()
    d_out = nc.dram_tensor("out", [U, S, H], F32, kind="ExternalOutput").ap()

    with tile.TileContext(nc) as tc:
        with (
            tc.tile_pool(name="const", bufs=1) as cpool,
            tc.tile_pool(name="work", bufs=3) as wpool,
            tc.tile_pool(name="sumt", bufs=3) as sumpool,
            tc.tile_pool(name="tanh", bufs=3) as tpool,
            tc.tile_pool(name="pers", bufs=1) as ppool,
            tc.tile_pool(name="gate", bufs=3) as gpool,
            tc.tile_pool(name="psA", bufs=2, space=bass.MemorySpace.PSUM) as psA,
            tc.tile_pool(name="psL", bufs=1, space=bass.MemorySpace.PSUM) as psL,
            tc.tile_pool(name="psR", bufs=2, space=bass.MemorySpace.PSUM) as psR,
        ):
            # ---- load constants / params into SBUF ----
            def load(pool, dram_ap, shape, dtype, tag):
                t = pool.tile(shape, dtype, tag=tag)
                nc.sync.dma_start(t[:], dram_ap)
                return t

            sb_x = [load(cpool, d_xbf[u], [S, D], BF, f"x{u}") for u in range(U)]
            sb_xt = [
                [load(cpool, d_xt[u, 128 * a : 128 * (a + 1), :], [128, S], BF,
                      f"xt{u}{a}")
                 for a in range(2)]
                for u in range(U)
            ]
            sb_w1 = [load(cpool, d_w1[128 * a : 128 * (a + 1), :], [128, D], BF,
                          f"w1{a}")
                     for a in range(2)]
            sb_w2 = [load(cpool, d_w2[128 * a : 128 * (a + 1), :], [128, D], BF,
                          f"w2{a}")
                     for a in range(2)]
            sb_biasv = load(cpool, d_biasv, [128, 2], F32, "biasv")
            sb_v3 = load(cpool, d_v3, [128, 2], BF, "v3")
            sb_gk = [load(cpool, d_gk[128 * k : 128 * (k + 1), :], [128, 3 * H], BF,
                          f"gk{k}")
                     for k in range(4)]
            sb_rk = [load(cpool, d_rk[128 * k : 128 * (k + 1), :], [128, 3 * H], BF,
                          f"rk{k}")
                     for k in range(2)]
            sb_btot = load(cpool, d_btot, [128, 8], F32, "btot")
            sb_ones = load(cpool, d_ones, [128, 1], BF, "ones")
            sb_idbf = load(cpool, d_idbf, [128, 128], BF, "idbf")
            sb_idf = load(cpool, d_idf, [128, 128], F32, "idf")

            # ---- S1: leftT (+bias) and rightT per (u, d-half e) ----
            leftrep = [[None] * 2 for _ in range(U)]  # [128, 256] m-duplicated
            rightT = [[None] * 2 for _ in range(U)]  # [128, 128]
            for u in range(U):
                for e in range(2):
                    psl = psA.tile([128, 128], F32, tag="ps")
                    for a in range(2):
                        nc.tensor.matmul(
                            psl[:],
                            sb_w1[a][:, 128 * e : 128 * (e + 1)],
                            sb_xt[u][a][:],
                            start=(a == 0),
                            stop=(a == 1),
                        )
                    # leftTb = psl + bias[:, e]  (bf16)
                    lb = wpool.tile([128, 128], BF, tag="leftb")
                    nc.vector.tensor_scalar(
                        lb[:], psl[:], sb_biasv[:, e : e + 1], None, ALU.add
                    )
                    # duplicate along free: leftrep[p, 2*m + j] = lb[p, m]
                    lr = ppool.tile([128, 256], BF, tag=f"leftrep_{u}_{e}")
                    nc.vector.tensor_copy(
                        lr[:].rearrange("p (m r) -> p m r", r=2),
                        lb[:].unsqueeze(2).to_broadcast((128, 128, 2)),
                    )
                    leftrep[u][e] = lr

                    psr = psA.tile([128, 128], F32, tag="ps")
                    for a in range(2):
                        nc.tensor.matmul(
                            psr[:],
                            sb_w2[a][:, 128 * e : 128 * (e + 1)],
                            sb_xt[u][a][:],
                            start=(a == 0),
                            stop=(a == 1),
                        )
                    rt = ppool.tile([128, 128], BF, tag=f"rightT_{u}_{e}")
                    nc.scalar.copy(rt[:], psr[:])
                    rightT[u][e] = rt

            # ---- S2+S3: SUM -> tanh -> logits (per u, m-chunk) ----
            logT_ps = []  # PSUM logitsT [n, m] per u
            for u in range(U):
                lp = psL.tile([128, 128], F32, tag=f"logT_{u}")
                logT_ps.append(lp)
            for u in range(U):
                for mc in range(NCHUNK):
                    tt = [None, None]
                    for e in range(2):
                        sm = sumpool.tile([128, MCH, 64, 2], BF, tag="sum")
                        in0 = (
                            rightT[u][e][:]
                            .rearrange("p (nh nl) -> p nh nl", nl=2)
                            .unsqueeze(1)
                            .to_broadcast((128, MCH, 64, 2))
                        )
                        in1 = (
                            leftrep[u][e][:]
                            .rearrange("p (m r) -> p m r", r=2)[
                                :, MCH * mc : MCH * (mc + 1), :
                            ]
                            .unsqueeze(2)
                            .to_broadcast((128, MCH, 64, 2))
                        )
                        nc.vector.tensor_tensor(sm[:], in0, in1, ALU.add)
                        th = tpool.tile([128, MCH, 64, 2], BF, tag="tanh")
                        nc.scalar.activation(th[:], sm[:], AF.Tanh)
                        tt[e] = th
                    for ml in range(MCH):
                        m = MCH * mc + ml
                        for e in range(2):
                            nc.tensor.matmul(
                                logT_ps[u][:, m : m + 1],
                                tt[e][:, ml],
                                sb_v3[:, e : e + 1],
                                start=(e == 0),
                                stop=(e == 1),
                            )

            # ---- S4: softmax (unnormalized) + context ----
            ct_sb = [[None] * 2 for _ in range(U)]  # cT bf16 [128 d-half, 128 m]
            for u in range(U):
                E = wpool.tile([128, 128], BF, tag="E")
                nc.scalar.activation(E[:], logT_ps[u][:], AF.Exp)
                cs = psA.tile([128, 1], F32, tag="ps")
                nc.tensor.matmul(cs[:], E[:], sb_ones[:])
                rs = wpool.tile([128, 1], F32, tag="rs")
                nc.vector.reciprocal(rs[:], cs[:])
                cps = psA.tile([128, 256], F32, tag="ps")
                nc.tensor.matmul(cps[:], E[:], sb_x[u][:])
                cn = wpool.tile([128, 256], BF, tag="cn")
                nc.vector.tensor_scalar(cn[:], cps[:], rs[:], None, ALU.mult)
                for e in range(2):
                    tp = psA.tile([128, 128], F32, tag="ps")
                    nc.tensor.matmul(
                        tp[:], cn[:, 128 * e : 128 * (e + 1)], sb_idbf[:]
                    )
                    ct = ppool.tile([128, 128], BF, tag=f"ct_{u}_{e}")
                    nc.scalar.copy(ct[:], tp[:])
                    ct_sb[u][e] = ct

            # ---- S5: xpT = (gk^T @ [x;c]T) + (b_in + b_rec), transposed ----
            # xpT[p, c6, u, t]
            xpT = ppool.tile([128, 6, U, S], BF, tag="xpT")
            for u in range(U):
                xtiles = [sb_xt[u][0], sb_xt[u][1], ct_sb[u][0], ct_sb[u][1]]
                for c in range(6):
                    ps = psA.tile([128, 128], F32, tag="ps")
                    for k in range(4):
                        nc.tensor.matmul(
                            ps[:],
                            sb_gk[k][:, 128 * c : 128 * (c + 1)],
                            xtiles[k][:],
                            start=(k == 0),
                            stop=(k == 3),
                        )
                    nc.scalar.activation(
                        xpT[:, c, u, :], ps[:], AF.Identity,
                        bias=sb_btot[:, c : c + 1],
                    )

            # ---- S6: GRU over 128 steps ----
            # h[p, k(half), u] bf16 ; hout[p, u, k, t] f32
            h_sb = ppool.tile([128, 2, U], BF, tag="h")
            nc.vector.memset(h_sb[:], 0.0)
            hout = ppool.tile([128, U, 2, S], F32, tag="hout")
            for t in range(S):
                ps = psR.tile([128, 12], F32, tag="rp")
                psv = ps[:].rearrange("p (c u) -> p c u", u=U)
                for c in range(6):
                    for k in range(2):
                        nc.tensor.matmul(
                            psv[:, c, :],
                            sb_rk[k][:, 128 * c : 128 * (c + 1)],
                            h_sb[:, k, :],
                            start=(k == 0),
                            stop=(k == 1),
                        )
                xz = xpT[:, 0:4, :, t : t + 1].squeeze(3)
                zr_pre = gpool.tile([128, 4, U], BF, tag="zrp")
                nc.vector.tensor_tensor(zr_pre[:], psv[:, 0:4, :], xz, ALU.add)
                zr = gpool.tile([128, 4, U], BF, tag="zr")
                nc.scalar.activation(zr[:], zr_pre[:], AF.Sigmoid)
                hh1 = gpool.tile([128, 2, U], BF, tag="hh1")
                for j in range(2):
                    nc.vector.scalar_tensor_tensor(
                        hh1[:, j, :], psv[:, 4 + j, :], sb_btot[:, 6 + j : 7 + j],
                        zr[:, 2 + j, :], ALU.add, ALU.mult,
                    )
                hh2 = gpool.tile([128, 2, U], BF, tag="hh2")
                nc.vector.tensor_tensor(
                    hh2[:], hh1[:], xpT[:, 4:6, :, t : t + 1].squeeze(3), ALU.add
                )
                hh = gpool.tile([128, 2, U], BF, tag="hh")
                nc.scalar.activation(hh[:], hh2[:], AF.Tanh)
                dm = gpool.tile([128, 2, U], BF, tag="dm")
                nc.vector.tensor_tensor(dm[:], h_sb[:], hh[:], ALU.subtract)
                ee = gpool.tile([128, 2, U], BF, tag="ee")
                nc.vector.tensor_tensor(ee[:], zr[:, 0:2, :], dm[:], ALU.mult)
                nc.vector.tensor_tensor(h_sb[:], hh[:], ee[:], ALU.add)
                # hout[p, u, k, t] = h[p, k, u]
                nc.scalar.copy(
                    hout[:, :, :, t : t + 1].squeeze(3),
                    h_sb[:].transpose([0, 2, 1]),
                )

            # ---- S7: transpose hout -> [t, H] and DMA out ----
            houtT = ppool.tile([128, U, 256], F32, tag="houtT")
            for u in range(U):
                for k in range(2):
                    tp = psA.tile([128, 128], F32, tag="ps")
                    nc.tensor.matmul(tp[:], hout[:, u, k, :], sb_idf[:])
                    nc.scalar.copy(houtT[:, u, 128 * k : 128 * (k + 1)], tp[:])
                nc.sync.dma_start(d_out[u], houtT[:, u, :])

    return nc


@functools.lru_cache(maxsize=1)
def _graph():
    return _build_graph()


def _host_prep(features, w1, w2, bias, v3, gru_kernel, gru_rec_kernel, gru_bias):
    x = np.ascontiguousarray(features.reshape(B * T, S, D)).astype(np.float32)
    params = {
        "w1bf": np.ascontiguousarray(w1.astype(BF_NP)),
        "w2bf": np.ascontiguousarray(w2.astype(BF_NP)),
        "biasv": np.ascontiguousarray(bias.reshape(2, 128).T.astype(np.float32)),
        "v3bf": np.ascontiguousarray(v3.reshape(2, 128).T.astype(BF_NP)),
        "gkbf": np.ascontiguousarray(gru_kernel.astype(BF_NP)),
        "rkbf": np.ascontiguousarray(gru_rec_kernel.astype(BF_NP)),
        "btot": np.ascontiguousarray(
            np.concatenate(
                [
                    np.concatenate(
                        [
                            (gru_bias[0] + gru_bias[1])[: 2 * 256],
                            gru_bias[0][2 * 256 :],
                        ]
                    ).reshape(6, 128).T,
                    gru_bias[1][2 * 256 :].reshape(2, 128).T,
                ],
                axis=1,
            ).astype(np.float32)
        ),
        "onesbf": np.ones((128, 1), dtype=BF_NP),
        "idbf": np.eye(128, dtype=np.float32).astype(BF_NP),
        "idf32": np.eye(128, dtype=np.float32),
    }
    in_maps = []
    for i in range(NCORES):
        xs = x[U * i : U * (i + 1)]
        m = dict(params)
        m["xbf"] = np.ascontiguousarray(xs.astype(BF_NP))
        m["xtbf"] = np.ascontiguousarray(xs.transpose(0, 2, 1).astype(BF_NP))
        in_maps.append(m)
    return in_maps


class _Runner:
    """Persistent jitted PJRT executor (mirrors bass2jax.run_bass_via_pjrt's
    multi-core path, but reuses the compiled executable across calls)."""

    def __init__(self, nc):
        import jax
        from jax.sharding import Mesh, PartitionSpec
        from jax.experimental.shard_map import shard_map
        from concourse import bass2jax as b2j
        from concourse import mybir as _mb

        b2j.install_neuronx_cc_hook()
        self.nc = nc
        in_names, out_names, out_avals, zero_outs = [], [], [], []
        partition_name = (
            nc.partition_id_tensor.name if nc.partition_id_tensor else None
        )
        for alloc in nc.m.functions[0].allocations:
            if not isinstance(alloc, _mb.MemoryLocationSet):
                continue
            name = alloc.memorylocations[0].name
            if alloc.kind == "ExternalInput":
                if name != partition_name:
                    in_names.append(name)
            elif alloc.kind == "ExternalOutput":
                shape = tuple(alloc.tensor_shape)
                dtype = _mb.dt.np(alloc.dtype)
                out_names.append(name)
                out_avals.append(jax.core.ShapedArray(shape, dtype))
                zero_outs.append(np.zeros(shape, dtype))
        self.in_names, self.out_names = in_names, out_names
        self.zero_outs = zero_outs
        n_params, n_outs = len(in_names), len(out_avals)
        all_names = tuple(in_names + out_names + (
            [partition_name] if partition_name else []
        ))

        def _body(*args):
            operands = list(args)
            if partition_name is not None:
                operands.append(b2j.partition_id_tensor())
            return tuple(
                b2j._bass_exec_p.bind(
                    *operands,
                    out_avals=tuple(out_avals),
                    in_names=all_names,
                    out_names=tuple(out_names),
                    lowering_input_output_aliases=(),
                    sim_require_finite=True,
                    sim_require_nnan=True,
                    nc=nc,
                )
            )

        devices = jax.devices()[:NCORES]
        mesh = Mesh(np.asarray(devices), ("core",))
        in_specs = (PartitionSpec("core"),) * (n_params + n_outs)
        out_specs = (PartitionSpec("core"),) * n_outs
        self._fn = jax.jit(
            shard_map(
                _body, mesh=mesh, in_specs=in_specs, out_specs=out_specs,
                check_rep=False,
            ),
            keep_unused=True,
        )
        self.n_params = n_params

    def __call__(self, in_maps):
        import jax
        concat_in = [
            np.concatenate([np.asarray(m[n]) for m in in_maps], axis=0)
            for n in self.in_names
        ]
        concat_zeros = [
            np.zeros((NCORES * z.shape[0], *z.shape[1:]), z.dtype)
            for z in self.zero_outs
        ]
        outs = self._fn(*concat_in, *concat_zeros)
        outs = [np.asarray(o) for o in outs]
        res = []
        for c in range(NCORES):
            res.append(
                {
                    n: o[c * (o.shape[0] // NCORES) : (c + 1) * (o.shape[0] // NCORES)]
                    for n, o in zip(self.out_names, outs)
                }
            )
        return res


@functools.lru_cache(maxsize=1)
def _runner():
    return _Runner(_graph())


def run(inputs, trace=False):
    in_maps = _host_prep(**inputs)
    results = _runner()(in_maps)
    outs = [np.asarray(results[i]["out"]) for i in range(NCORES)]
    full = np.concatenate(outs, axis=0).reshape(B, T, S, H).astype(np.float32)
    return full, results


def kernel(**inputs) -> np.ndarray:
    out, _ = run(inputs, trace=False)
    return out


# revision 10
# speedup vs baseline: 111.5911x; 111.5911x over previous
"""Trainium2 Bass kernel for nn_AttentionFlow (additive attention + Keras GRU).

Data-parallel over 8 NeuronCores: B*T = 16 independent utterances, 2 per core.
Per utterance (x in [S=128, D=256]):
  left  = x @ w1 ; right = x @ w2
  logits[m,n] = sum_d v3[d] * tanh(left[m,d] + right[n,d] + bias[d])
  score = softmax(logits, axis=n) ; c = score @ x
  p = KerasGRU([x, c]) with h0 = 0  (reset_after=True)

Device layout choices (per core, 2 utterances):
  - SUM build on DVE in [d_half_p, (m, n)] layout, bf16 2x mode via a
    duplicated-left trick (left stored twice along free so innermost stride=1).
  - tanh on ACT with large free dims.
  - logits on PE: stationary = contiguous tanh tile [d_half, 128] per m (FWL),
    moving = v3 half column -> PSUM logitsT [n, m].
  - softmax normalization folded into c: c_un = E^T-matmul, then per-partition
    (per-m) scale by 1/rowsum.
  - GRU: R chunks stationary [128,128] (12 per step), moving = h [128, 2],
    output rp^T in [3H-chunk partitions, (chunk,u)] so gate math runs on
    [128, few]-tiles; h kept as [128, (k, u)] bf16.
"""

import functools
import sys

import numpy as np

sys.path.insert(0, "/opt/trn_rl_repo")

import ml_dtypes  # noqa: E402

import bass_rust  # noqa: E402
import concourse.bass as bass  # noqa: E402
import concourse.tile as tile  # noqa: E402
from concourse import mybir  # noqa: E402
from concourse.tile_scheduler import N_PROCS  # noqa: E402
from concourse.vector_clock import ScopedClock, VectorClock  # noqa: E402

BF = mybir.dt.bfloat16
F32 = mybir.dt.float32
AF = mybir.ActivationFunctionType
ALU = mybir.AluOpType
BF_NP = ml_dtypes.bfloat16

B, T, S, D = 4, 4, 128, 256
H = D
NCORES = 8
U = 2  # utterances per core
MCH = 32  # m-chunk size for the SUM/tanh/logits pipeline
NCHUNK = S // MCH


# ---------------------------------------------------------------------------
# Workarounds for this walrus build: (1) instructions may carry at most ONE
# sync wait ("Too many sync wait commands"); (2) the Tile kernel-tail drain
# aggregates one wait per logical proc onto a single drain.
# ---------------------------------------------------------------------------
def _patched_drain_and_barrier(self, tick_clock, wait_clock):
    g = tick_clock.global_clock
    for p in range(N_PROCS):
        try:
            v = g[p]
        except Exception:
            v = 0
        if v <= 0:
            continue
        onehot = VectorClock([g[q] if q == p else 0 for q in range(N_PROCS)])
        di = self.nc.sync.drain()
        wait_clock.add_sem_waits(di.ins, ScopedClock({None: onehot}))
    self.nc.all_engine_barrier()
    popped = self.nc._tile_sem_poison_stack.pop()
    assert popped is self._sem_poison
    self.nc.clear_and_free_semaphores(list(self.sems.allocated().values()))
    self.nc.all_engine_barrier()


tile.TileContext._drain_and_barrier = _patched_drain_and_barrier

_wsplit_counter = [0]
_orig_add_instruction = tile.TileContext._add_instruction


def _patched_add_instruction(self, inst):
    si = inst.sync_info
    waits = list(si.on_wait) if si and si.on_wait else []
    if len(waits) > 1:
        for w in waits[:-1]:
            _wsplit_counter[0] += 1
            c = mybir.InstEventSemaphore.__new__(mybir.InstEventSemaphore)
            c.name = f"wsplit_{_wsplit_counter[0]}"
            c.engine = inst.engine
            c.sync_info = bass_rust.SyncInfo(on_wait=[w], on_update=[])
            _orig_add_instruction(self, c)
        inst.sync_info = bass_rust.SyncInfo(
            on_wait=[waits[-1]], on_update=list(si.on_update or [])
        )
    _orig_add_instruction(self, inst)


tile.TileContext._add_instruction = _patched_add_instruction


# ---------------------------------------------------------------------------
# Graph
# ---------------------------------------------------------------------------
def _body(nc, pools, sb, rep):
    """One full problem body (both utterances). rep only disambiguates reuse."""
    cpool, wpool, sumpool, tpool, ppool, gpool, psA, psL, psR = pools

    # ---- S1: leftT (+bias) and rightT per (u, d-half e) ----
    leftrep = [[None] * 2 for _ in range(U)]
    rightT = [[None] * 2 for _ in range(U)]
    for u in range(U):
        for e in range(2):
            psl = psA.tile([128, 128], F32, tag="ps")
            for a in range(2):
                nc.tensor.matmul(
                    psl[:],
                    sb["w1"][a][:, 128 * e : 128 * (e + 1)],
                    sb["xt"][u][a][:],
                    start=(a == 0),
                    stop=(a == 1),
                )
            lb = wpool.tile([128, 128], BF, tag="leftb")
            nc.vector.tensor_scalar(
                lb[:], psl[:], sb["biasv"][:, e : e + 1], None, ALU.add
            )
            lr = ppool.tile([128, 256], BF, tag=f"leftrep_{u}_{e}")
            nc.vector.tensor_copy(
                lr[:].rearrange("p (m r) -> p m r", r=2),
                lb[:].unsqueeze(2).to_broadcast((128, 128, 2)),
            )
            leftrep[u][e] = lr

            psr = psA.tile([128, 128], F32, tag="ps")
            for a in range(2):
                nc.tensor.matmul(
                    psr[:],
                    sb["w2"][a][:, 128 * e : 128 * (e + 1)],
                    sb["xt"][u][a][:],
                    start=(a == 0),
                    stop=(a == 1),
                )
            rt = ppool.tile([128, 128], BF, tag=f"rightT_{u}_{e}")
            nc.scalar.copy(rt[:], psr[:])
            rightT[u][e] = rt

    # ---- S2+S3: SUM -> tanh -> logits ----
    logT_ps = []
    for u in range(U):
        lp = psL.tile([128, 128], F32, tag=f"logT_{u}")
        logT_ps.append(lp)
    for u in range(U):
        for mc in range(NCHUNK):
            tt = [None, None]
            for e in range(2):
                sm = sumpool.tile([128, MCH, 64, 2], BF, tag="sum")
                in0 = (
                    rightT[u][e][:]
                    .rearrange("p (nh nl) -> p nh nl", nl=2)
                    .unsqueeze(1)
                    .to_broadcast((128, MCH, 64, 2))
                )
                in1 = (
                    leftrep[u][e][:]
                    .rearrange("p (m r) -> p m r", r=2)[
                        :, MCH * mc : MCH * (mc + 1), :
                    ]
                    .unsqueeze(2)
                    .to_broadcast((128, MCH, 64, 2))
                )
                nc.vector.tensor_tensor(sm[:], in0, in1, ALU.add)
                th = tpool.tile([128, MCH, 64, 2], BF, tag="tanh")
                nc.scalar.activation(th[:], sm[:], AF.Tanh)
                tt[e] = th
            for ml in range(MCH):
                m = MCH * mc + ml
                for e in range(2):
                    nc.tensor.matmul(
                        logT_ps[u][:, m : m + 1],
                        tt[e][:, ml],
                        sb["v3"][:, e : e + 1],
                        start=(e == 0),
                        stop=(e == 1),
                    )

    # ---- S4: softmax (unnormalized) + context ----
    ct_sb = [[None] * 2 for _ in range(U)]
    for u in range(U):
        E = wpool.tile([128, 128], BF, tag="E")
        nc.scalar.activation(E[:], logT_ps[u][:], AF.Exp)
        cs = psA.tile([128, 1], F32, tag="ps")
        nc.tensor.matmul(cs[:], E[:], sb["ones"][:])
        rs = wpool.tile([128, 1], F32, tag="rs")
        nc.vector.reciprocal(rs[:], cs[:])
        cps = psA.tile([128, 256], F32, tag="ps")
        nc.tensor.matmul(cps[:], E[:], sb["x"][u][:])
        cn = wpool.tile([128, 256], BF, tag="cn")
        nc.vector.tensor_scalar(cn[:], cps[:], rs[:], None, ALU.mult)
        for e in range(2):
            tp = psA.tile([128, 128], F32, tag="ps")
            nc.tensor.matmul(tp[:], cn[:, 128 * e : 128 * (e + 1)], sb["idbf"][:])
            ct = ppool.tile([128, 128], BF, tag=f"ct_{u}_{e}")
            nc.scalar.copy(ct[:], tp[:])
            ct_sb[u][e] = ct

    # ---- S5: xpT = (gk^T @ [x;c]T) + biases, transposed ----
    xpT = ppool.tile([128, 6, U, S], BF, tag="xpT")
    for u in range(U):
        xtiles = [sb["xt"][u][0], sb["xt"][u][1], ct_sb[u][0], ct_sb[u][1]]
        for c in range(6):
            ps = psA.tile([128, 128], F32, tag="ps")
            for k in range(4):
                nc.tensor.matmul(
                    ps[:],
                    sb["gk"][k][:, 128 * c : 128 * (c + 1)],
                    xtiles[k][:],
                    start=(k == 0),
                    stop=(k == 3),
                )
            nc.scalar.activation(
                xpT[:, c, u, :], ps[:], AF.Identity, bias=sb["btot"][:, c : c + 1]
            )

    # ---- S6: GRU over S steps ----
    h_sb = ppool.tile([128, 2, U], BF, tag="h")
    nc.vector.memset(h_sb[:], 0.0)
    hout = ppool.tile([128, U, 2, S], F32, tag="hout")
    for t in range(S):
        ps = psR.tile([128, 12], F32, tag="rp")
        psv = ps[:].rearrange("p (c u) -> p c u", u=U)
        for c in range(6):
            for k in range(2):
                nc.tensor.matmul(
                    psv[:, c, :],
                    sb["rk"][k][:, 128 * c : 128 * (c + 1)],
                    h_sb[:, k, :],
                    start=(k == 0),
                    stop=(k == 1),
                )
        xz = xpT[:, 0:4, :, t : t + 1].squeeze(3)
        zr_pre = gpool.tile([128, 4, U], BF, tag="zrp")
        nc.vector.tensor_tensor(zr_pre[:], psv[:, 0:4, :], xz, ALU.add)
        zr = gpool.tile([128, 4, U], BF, tag="zr")
        nc.scalar.activation(zr[:], zr_pre[:], AF.Sigmoid)
        hh1 = gpool.tile([128, 2, U], BF, tag="hh1")
        for j in range(2):
            nc.vector.scalar_tensor_tensor(
                hh1[:, j, :], psv[:, 4 + j, :], sb["btot"][:, 6 + j : 7 + j],
                zr[:, 2 + j, :], ALU.add, ALU.mult,
            )
        hh2 = gpool.tile([128, 2, U], BF, tag="hh2")
        nc.vector.tensor_tensor(
            hh2[:], hh1[:], xpT[:, 4:6, :, t : t + 1].squeeze(3), ALU.add
        )
        hh = gpool.tile([128, 2, U], BF, tag="hh")
        nc.scalar.activation(hh[:], hh2[:], AF.Tanh)
        dm = gpool.tile([128, 2, U], BF, tag="dm")
        nc.vector.tensor_tensor(dm[:], h_sb[:], hh[:], ALU.subtract)
        ee = gpool.tile([128, 2, U], BF, tag="ee")
        nc.vector.tensor_tensor(ee[:], zr[:, 0:2, :], dm[:], ALU.mult)
        nc.vector.tensor_tensor(h_sb[:], hh[:], ee[:], ALU.add)
        nc.scalar.copy(
            hout[:, :, :, t : t + 1].squeeze(3), h_sb[:].transpose([0, 2, 1])
        )

    # ---- S7: transpose hout -> [t, H] ----
    houtT = ppool.tile([128, U, 256], F32, tag="houtT")
    for u in range(U):
        for k in range(2):
            tp = psA.tile([128, 128], F32, tag="ps")
            nc.tensor.matmul(tp[:], hout[:, u, k, :], sb["idf"][:])
            nc.scalar.copy(houtT[:, u, 128 * k : 128 * (k + 1)], tp[:])
    return houtT


def _build_graph(reps=1):
    nc = bass.Bass("TRN2", target_bir_lowering=False, debug=False)

    d_xbf = nc.dram_tensor("xbf", [U, S, D], BF, kind="ExternalInput").ap()
    d_xt = nc.dram_tensor("xtbf", [U, D, S], BF, kind="ExternalInput").ap()
    d_w1 = nc.dram_tensor("w1bf", [D, D], BF, kind="ExternalInput").ap()
    d_w2 = nc.dram_tensor("w2bf", [D, D], BF, kind="ExternalInput").ap()
    d_biasv = nc.dram_tensor("biasv", [128, 2], F32, kind="ExternalInput").ap()
    d_v3 = nc.dram_tensor("v3bf", [128, 2], BF, kind="ExternalInput").ap()
    d_gk = nc.dram_tensor("gkbf", [2 * D, 3 * H], BF, kind="ExternalInput").ap()
    d_rk = nc.dram_tensor("rkbf", [H, 3 * H], BF, kind="ExternalInput").ap()
    d_btot = nc.dram_tensor("btot", [128, 8], F32, kind="ExternalInput").ap()
    d_ones = nc.dram_tensor("onesbf", [128, 1], BF, kind="ExternalInput").ap()
    d_idbf = nc.dram_tensor("idbf", [128, 128], BF, kind="ExternalInput").ap()
    d_idf = nc.dram_tensor("idf32", [128, 128], F32, kind="ExternalInput").ap()
    d_out = nc.dram_tensor("out", [U, S, H], F32, kind="ExternalOutput").ap()

    with tile.TileContext(nc) as tc:
        with (
            tc.tile_pool(name="const", bufs=1) as cpool,
            tc.tile_pool(name="work", bufs=3) as wpool,
            tc.tile_pool(name="sumt", bufs=3) as sumpool,
            tc.tile_pool(name="tanh", bufs=3) as tpool,
            tc.tile_pool(name="pers", bufs=1) as ppool,
            tc.tile_pool(name="gate", bufs=3) as gpool,
            tc.tile_pool(name="psA", bufs=2, space=bass.MemorySpace.PSUM) as psA,
            tc.tile_pool(name="psL", bufs=1, space=bass.MemorySpace.PSUM) as psL,
            tc.tile_pool(name="psR", bufs=2, space=bass.MemorySpace.PSUM) as psR,
        ):
            pools = (cpool, wpool, sumpool, tpool, ppool, gpool, psA, psL, psR)

            def load(dram_ap, shape, dtype, tag):
                t = cpool.tile(shape, dtype, tag=tag)
                nc.sync.dma_start(t[:], dram_ap)
                return t

            sb = {
                "x": [load(d_xbf[u], [S, D], BF, f"x{u}") for u in range(U)],
                "xt": [
                    [
                        load(
                            d_xt[u, 128 * a : 128 * (a + 1), :],
                            [128, S], BF, f"xt{u}{a}",
                        )
                        for a in range(2)
                    ]
                    for u in range(U)
                ],
                "w1": [
                    load(d_w1[128 * a : 128 * (a + 1), :], [128, D], BF, f"w1{a}")
                    for a in range(2)
                ],
                "w2": [
                    load(d_w2[128 * a : 128 * (a + 1), :], [128, D], BF, f"w2{a}")
                    for a in range(2)
                ],
                "biasv": load(d_biasv, [128, 2], F32, "biasv"),
                "v3": load(d_v3, [128, 2], BF, "v3"),
                "gk": [
                    load(d_gk[128 * k : 128 * (k + 1), :], [128, 3 * H], BF, f"gk{k}")
                    for k in range(4)
                ],
                "rk": [
                    load(d_rk[128 * k : 128 * (k + 1), :], [128, 3 * H], BF, f"rk{k}")
                    for k in range(2)
                ],
                "btot": load(d_btot, [128, 8], F32, "btot"),
                "ones": load(d_ones, [128, 1], BF, "ones"),
                "idbf": load(d_idbf, [128, 128], BF, "idbf"),
                "idf": load(d_idf, [128, 128], F32, "idf"),
            }

            houtT = None
            for rep in range(reps):
                houtT = _body(nc, pools, sb, rep)
            for u in range(U):
                nc.sync.dma_start(d_out[u], houtT[:, u, :])

    return nc


@functools.lru_cache(maxsize=2)
def _graph(reps=1):
    return _build_graph(reps)


def _host_prep(features, w1, w2, bias, v3, gru_kernel, gru_rec_kernel, gru_bias):
    x = np.ascontiguousarray(np.asarray(features, np.float32).reshape(B * T, S, D))
    gb = np.asarray(gru_bias, np.float32)
    btot_main = (
        np.concatenate([(gb[0] + gb[1])[: 2 * H], gb[0][2 * H :]]).reshape(6, 128).T
    )
    btot = np.concatenate(
        [btot_main, gb[1][2 * H :].reshape(2, 128).T], axis=1
    ).astype(np.float32)
    params = {
        "w1bf": np.ascontiguousarray(np.asarray(w1).astype(BF_NP)),
        "w2bf": np.ascontiguousarray(np.asarray(w2).astype(BF_NP)),
        "biasv": np.ascontiguousarray(
            np.asarray(bias, np.float32).reshape(2, 128).T
        ),
        "v3bf": np.ascontiguousarray(
            np.asarray(v3, np.float32).reshape(2, 128).T.astype(BF_NP)
        ),
        "gkbf": np.ascontiguousarray(np.asarray(gru_kernel).astype(BF_NP)),
        "rkbf": np.ascontiguousarray(np.asarray(gru_rec_kernel).astype(BF_NP)),
        "btot": np.ascontiguousarray(btot),
        "onesbf": np.ones((128, 1), dtype=BF_NP),
        "idbf": np.eye(128, dtype=np.float32).astype(BF_NP),
        "idf32": np.eye(128, dtype=np.float32),
    }
    in_maps = []
    for i in range(NCORES):
        xs = x[U * i : U * (i + 1)]
        m = dict(params)
        m["xbf"] = np.ascontiguousarray(xs.astype(BF_NP))
        m["xtbf"] = np.ascontiguousarray(xs.transpose(0, 2, 1).astype(BF_NP))
        in_maps.append(m)
    return in_maps


class _Runner:
    """Persistent jitted PJRT executor (mirrors bass2jax.run_bass_via_pjrt's
    multi-core path, but reuses the compiled executable across calls)."""

    def __init__(self, nc):
        import jax
        from jax.experimental.shard_map import shard_map
        from jax.sharding import Mesh, PartitionSpec

        from concourse import bass2jax as b2j
        from concourse import mybir as _mb

        b2j.install_neuronx_cc_hook()
        self.nc = nc
        in_names, out_names, out_avals, zero_outs = [], [], [], []
        partition_name = (
            nc.partition_id_tensor.name if nc.partition_id_tensor else None
        )
        for alloc in nc.m.functions[0].allocations:
            if not isinstance(alloc, _mb.MemoryLocationSet):
                continue
            name = alloc.memorylocations[0].name
            if alloc.kind == "ExternalInput":
                if name != partition_name:
                    in_names.append(name)
            elif alloc.kind == "ExternalOutput":
                shape = tuple(alloc.tensor_shape)
                dtype = _mb.dt.np(alloc.dtype)
                out_names.append(name)
                out_avals.append(jax.core.ShapedArray(shape, dtype))
                zero_outs.append(np.zeros(shape, dtype))
        self.in_names, self.out_names = in_names, out_names
        self.zero_outs = zero_outs
        n_params, n_outs = len(in_names), len(out_avals)
        all_names = tuple(
            in_names + out_names + ([partition_name] if partition_name else [])
        )

        def _bb(*args):
            operands = list(args)
            if partition_name is not None:
                operands.append(b2j.partition_id_tensor())
            return tuple(
                b2j._bass_exec_p.bind(
                    *operands,
                    out_avals=tuple(out_avals),
                    in_names=all_names,
                    out_names=tuple(out_names),
                    lowering_input_output_aliases=(),
                    sim_require_finite=True,
                    sim_require_nnan=True,
                    nc=nc,
                )
            )

        devices = jax.devices()[:NCORES]
        mesh = Mesh(np.asarray(devices), ("core",))
        in_specs = (PartitionSpec("core"),) * (n_params + n_outs)
        out_specs = (PartitionSpec("core"),) * n_outs
        self._fn = jax.jit(
            shard_map(
                _bb, mesh=mesh, in_specs=in_specs, out_specs=out_specs,
                check_rep=False,
            ),
            keep_unused=True,
        )

    def __call__(self, in_maps):
        concat_in = [
            np.concatenate([np.asarray(m[n]) for m in in_maps], axis=0)
            for n in self.in_names
        ]
        concat_zeros = [
            np.zeros((NCORES * z.shape[0], *z.shape[1:]), z.dtype)
            for z in self.zero_outs
        ]
        outs = [np.asarray(o) for o in self._fn(*concat_in, *concat_zeros)]
        res = []
        for c in range(NCORES):
            res.append(
                {
                    n: o[c * (o.shape[0] // NCORES) : (c + 1) * (o.shape[0] // NCORES)]
                    for n, o in zip(self.out_names, outs)
                }
            )
        return res


@functools.lru_cache(maxsize=2)
def _runner(reps=1):
    return _Runner(_graph(reps))


def run(inputs, reps=1):
    in_maps = _host_prep(**inputs)
    results = _runner(reps)(in_maps)
    outs = [np.asarray(results[i]["out"]) for i in range(NCORES)]
    full = np.concatenate(outs, axis=0).reshape(B, T, S, H).astype(np.float32)
    return full, results


def kernel(**inputs) -> np.ndarray:
    out, _ = run(inputs)
    return out


# revision 15
# speedup vs baseline: 170.1657x; 1.5249x over previous
"""Trainium2 Bass kernel for nn_AttentionFlow (additive attention + Keras GRU).

Data-parallel over 8 NeuronCores: B*T = 16 independent utterances, 2 per core.
Per utterance (x in [S=128, D=256]):
  left  = x @ w1 ; right = x @ w2
  logits[m,n] = sum_d v3[d] * tanh(left[m,d] + right[n,d] + bias[d])
  score = softmax(logits, axis=n) ; c = score @ x
  p = KerasGRU([x, c]) with h0 = 0  (reset_after=True)

Device layout choices (per core, 2 utterances):
  - SUM build on DVE in [d_half_p, (m, n)] layout, bf16 2x mode via a
    duplicated-left trick (left stored twice along free so innermost stride=1).
  - tanh on ACT with large free dims.
  - logits on PE: stationary = contiguous tanh tile [d_half, 128] per m (FWL),
    moving = v3 half column -> PSUM logitsT [n, m].
  - softmax normalization folded into c: c_un = E^T-matmul, then per-partition
    (per-m) scale by 1/rowsum.
  - GRU: R chunks stationary [128,128] (12 per step), moving = h [128, 2],
    output rp^T in [3H-chunk partitions, (chunk,u)] so gate math runs on
    [128, few]-tiles; h kept as [128, (k, u)] bf16.
"""

import functools
import sys

import numpy as np

sys.path.insert(0, "/opt/trn_rl_repo")

import ml_dtypes  # noqa: E402

import bass_rust  # noqa: E402
import concourse.bass as bass  # noqa: E402
import concourse.tile as tile  # noqa: E402
from concourse import mybir  # noqa: E402
from concourse.tile_scheduler import N_PROCS  # noqa: E402
from concourse.vector_clock import ScopedClock, VectorClock  # noqa: E402

BF = mybir.dt.bfloat16
F32 = mybir.dt.float32
AF = mybir.ActivationFunctionType
ALU = mybir.AluOpType
BF_NP = ml_dtypes.bfloat16

B, T, S, D = 4, 4, 128, 256
H = D
NCORES = 8
U = 2  # utterances per core
MCH = 32  # m-chunk size for the SUM/tanh/logits pipeline
NCHUNK = S // MCH


# ---------------------------------------------------------------------------
# Workarounds for this walrus build: (1) instructions may carry at most ONE
# sync wait ("Too many sync wait commands"); (2) the Tile kernel-tail drain
# aggregates one wait per logical proc onto a single drain.
# ---------------------------------------------------------------------------
def _patched_drain_and_barrier(self, tick_clock, wait_clock):
    g = tick_clock.global_clock
    for p in range(N_PROCS):
        try:
            v = g[p]
        except Exception:
            v = 0
        if v <= 0:
            continue
        onehot = VectorClock([g[q] if q == p else 0 for q in range(N_PROCS)])
        di = self.nc.sync.drain()
        wait_clock.add_sem_waits(di.ins, ScopedClock({None: onehot}))
    self.nc.all_engine_barrier()
    popped = self.nc._tile_sem_poison_stack.pop()
    assert popped is self._sem_poison
    self.nc.clear_and_free_semaphores(list(self.sems.allocated().values()))
    self.nc.all_engine_barrier()


tile.TileContext._drain_and_barrier = _patched_drain_and_barrier

_wsplit_counter = [0]
_orig_add_instruction = tile.TileContext._add_instruction


def _patched_add_instruction(self, inst):
    si = inst.sync_info
    waits = list(si.on_wait) if si and si.on_wait else []
    if len(waits) > 1:
        for w in waits[:-1]:
            _wsplit_counter[0] += 1
            c = mybir.InstEventSemaphore.__new__(mybir.InstEventSemaphore)
            c.name = f"wsplit_{_wsplit_counter[0]}"
            c.engine = inst.engine
            c.sync_info = bass_rust.SyncInfo(on_wait=[w], on_update=[])
            _orig_add_instruction(self, c)
        inst.sync_info = bass_rust.SyncInfo(
            on_wait=[waits[-1]], on_update=list(si.on_update or [])
        )
    _orig_add_instruction(self, inst)


tile.TileContext._add_instruction = _patched_add_instruction


# ---------------------------------------------------------------------------
# Graph
# ---------------------------------------------------------------------------
def _body(nc, pools, sb, rep):
    """One full problem body (both utterances). rep only disambiguates reuse."""
    cpool, wpool, sumpool, tpool, ppool, gpool, psA, psL, psR = pools

    # ---- S1: leftb [m,d] (+bias) and right [n,d] ----
    import os
    skip_sum = os.environ.get("K_SKIP_SUM", "0") == "1"
    skip_logits = os.environ.get("K_SKIP_LOGITS", "0") == "1"
    leftb = [None] * U
    right = [None] * U
    for u in range(U):
        psl = psA.tile([128, 256], F32, tag="ps")
        for a in range(2):
            nc.tensor.matmul(
                psl[:], sb["xt"][u][a][:], sb["w1"][a][:],
                start=(a == 0), stop=False,
            )
        nc.tensor.matmul(
            psl[:], sb["onesrow"][:], sb["biasrow"][:], start=False, stop=True
        )
        lb = ppool.tile([128, 256], BF, tag=f"leftb_{u}")
        nc.scalar.copy(lb[:], psl[:])
        leftb[u] = lb
        psr = psA.tile([128, 256], F32, tag="ps")
        for a in range(2):
            nc.tensor.matmul(
                psr[:], sb["xt"][u][a][:], sb["w2"][a][:],
                start=(a == 0), stop=(a == 1),
            )
        rt = ppool.tile([128, 256], BF, tag=f"right_{u}")
        nc.scalar.copy(rt[:], psr[:])
        right[u] = rt

    # ---- S2+S3: pair-layout SUM -> tanh -> (*v3) -> reduce_d -> logitsT ----
    # Chunk m into MCH-sized groups. LeftRep[n_p, (m_chunk, d)] is leftb
    # broadcast across partitions via DMA (idle engines), so the DVE add and
    # mul run in bf16 2x mode and the d-reduce is a single axis=X reduce.
    logT = []
    for u in range(U):
        lt = ppool.tile([128, 128], F32, tag=f"logT_{u}")
        logT.append(lt)
    # bounce leftb through DRAM so the replication DMA can use stride-0 dims
    for u in range(U):
        nc.sync.dma_start(sb["lbd"][u], leftb[u][:])
    for u in range(U):
        for mc in range(NCHUNK):
            lrep = sumpool.tile([128, MCH, 256], BF, tag="lrep")
            nc.sync.dma_start(
                lrep[:],
                sb["lbd"][u][MCH * mc : MCH * (mc + 1), :]
                .unsqueeze(0)
                .to_broadcast((128, MCH, 256)),
            )
            th = tpool.tile([128, MCH, 256], BF, tag="tanh")
            if not skip_sum:
                nc.vector.tensor_tensor(
                    th[:],
                    lrep[:],
                    right[u][:].unsqueeze(1).to_broadcast((128, MCH, 256)),
                    ALU.add,
                )
                nc.scalar.activation(th[:], th[:], AF.Tanh)
            if not skip_logits:
                nc.vector.tensor_tensor(
                    th[:],
                    th[:],
                    sb["v3rep"][:].unsqueeze(1).to_broadcast((128, MCH, 256)),
                    ALU.mult,
                )
                nc.vector.tensor_reduce(
                    logT[u][:, MCH * mc : MCH * (mc + 1)], th[:],
                    mybir.AxisListType.X, ALU.add,
                )

    # ---- S4: softmax (unnormalized) + context ----
    ct_sb = [[None] * 2 for _ in range(U)]
    for u in range(U):
        E = wpool.tile([128, 128], BF, tag="E")
        if skip_logits:
            nc.vector.memset(E[:], 0.01)
        else:
            nc.scalar.activation(E[:], logT[u][:], AF.Exp)
        cs = psA.tile([128, 1], F32, tag="ps")
        nc.tensor.matmul(cs[:], E[:], sb["ones"][:])
        rs = wpool.tile([128, 1], F32, tag="rs")
        nc.vector.reciprocal(rs[:], cs[:])
        cps = psA.tile([128, 256], F32, tag="ps")
        nc.tensor.matmul(cps[:], E[:], sb["x"][u][:])
        cn = wpool.tile([128, 256], BF, tag="cn")
        nc.vector.tensor_scalar(cn[:], cps[:], rs[:], None, ALU.mult)
        for e in range(2):
            tp = psA.tile([128, 128], F32, tag="ps")
            nc.tensor.matmul(tp[:], cn[:, 128 * e : 128 * (e + 1)], sb["idbf"][:])
            ct = ppool.tile([128, 128], BF, tag=f"ct_{u}_{e}")
            nc.scalar.copy(ct[:], tp[:])
            ct_sb[u][e] = ct

    # ---- S5: xpT = (gk^T @ [x;c]T) + biases, transposed ----
    xpT = ppool.tile([128, 6, U, S], BF, tag="xpT")
    for u in range(U):
        xtiles = [sb["xt"][u][0], sb["xt"][u][1], ct_sb[u][0], ct_sb[u][1]]
        for c in range(6):
            ps = psA.tile([128, 128], F32, tag="ps")
            for k in range(4):
                nc.tensor.matmul(
                    ps[:],
                    sb["gk"][k][:, 128 * c : 128 * (c + 1)],
                    xtiles[k][:],
                    start=(k == 0),
                    stop=(k == 3),
                )
            nc.scalar.activation(
                xpT[:, c, u, :], ps[:], AF.Identity, bias=sb["btot"][:, c : c + 1]
            )

    # ---- S6: GRU over S steps ----
    import os
    gru_steps = int(os.environ.get("K_GRU_STEPS", S))
    h_sb = ppool.tile([128, 2, U], BF, tag="h")
    nc.vector.memset(h_sb[:], 0.0)
    hout = ppool.tile([128, U, 2, S], F32, tag="hout")
    nc.vector.memset(hout[:], 0.0)
    for t in range(gru_steps):
        ps = psR.tile([128, 12], F32, tag="rp")
        psv = ps[:].rearrange("p (c u) -> p c u", u=U)
        for c in range(6):
            for k in range(2):
                nc.tensor.matmul(
                    psv[:, c, :],
                    sb["rk"][k][:, 128 * c : 128 * (c + 1)],
                    h_sb[:, k, :],
                    start=(k == 0),
                    stop=(k == 1),
                )
        xz = xpT[:, 0:4, :, t : t + 1].squeeze(3)
        zr_pre = gpool.tile([128, 4, U], BF, tag="zrp")
        nc.vector.tensor_tensor(zr_pre[:], psv[:, 0:4, :], xz, ALU.add)
        zr = gpool.tile([128, 4, U], BF, tag="zr")
        nc.scalar.activation(zr[:], zr_pre[:], AF.Sigmoid)
        hh1 = gpool.tile([128, 2, U], BF, tag="hh1")
        for j in range(2):
            nc.vector.scalar_tensor_tensor(
                hh1[:, j, :], psv[:, 4 + j, :], sb["btot"][:, 6 + j : 7 + j],
                zr[:, 2 + j, :], ALU.add, ALU.mult,
            )
        hh2 = gpool.tile([128, 2, U], BF, tag="hh2")
        nc.vector.tensor_tensor(
            hh2[:], hh1[:], xpT[:, 4:6, :, t : t + 1].squeeze(3), ALU.add
        )
        hh = gpool.tile([128, 2, U], BF, tag="hh")
        nc.scalar.activation(hh[:], hh2[:], AF.Tanh)
        dm = gpool.tile([128, 2, U], BF, tag="dm")
        nc.vector.tensor_tensor(dm[:], h_sb[:], hh[:], ALU.subtract)
        ee = gpool.tile([128, 2, U], BF, tag="ee")
        nc.vector.tensor_tensor(ee[:], zr[:, 0:2, :], dm[:], ALU.mult)
        nc.vector.tensor_tensor(h_sb[:], hh[:], ee[:], ALU.add)
        nc.scalar.copy(
            hout[:, :, :, t : t + 1].squeeze(3), h_sb[:].transpose([0, 2, 1])
        )

    # ---- S7: transpose hout -> [t, H] ----
    houtT = ppool.tile([128, U, 256], F32, tag="houtT")
    for u in range(U):
        for k in range(2):
            tp = psA.tile([128, 128], F32, tag="ps")
            nc.tensor.matmul(tp[:], hout[:, u, k, :], sb["idf"][:])
            nc.scalar.copy(houtT[:, u, 128 * k : 128 * (k + 1)], tp[:])
    return houtT


def _build_graph(reps=1):
    nc = bass.Bass("TRN2", target_bir_lowering=False, debug=False)

    d_xbf = nc.dram_tensor("xbf", [U, S, D], BF, kind="ExternalInput").ap()
    d_xt = nc.dram_tensor("xtbf", [U, D, S], BF, kind="ExternalInput").ap()
    d_w1 = nc.dram_tensor("w1bf", [D, D], BF, kind="ExternalInput").ap()
    d_w2 = nc.dram_tensor("w2bf", [D, D], BF, kind="ExternalInput").ap()
    d_biasv = nc.dram_tensor("biasv", [128, 2], F32, kind="ExternalInput").ap()
    d_v3 = nc.dram_tensor("v3bf", [128, 2], BF, kind="ExternalInput").ap()
    d_gk = nc.dram_tensor("gkbf", [2 * D, 3 * H], BF, kind="ExternalInput").ap()
    d_rk = nc.dram_tensor("rkbf", [H, 3 * H], BF, kind="ExternalInput").ap()
    d_btot = nc.dram_tensor("btot", [128, 8], F32, kind="ExternalInput").ap()
    d_ones = nc.dram_tensor("onesbf", [128, 1], BF, kind="ExternalInput").ap()
    d_idbf = nc.dram_tensor("idbf", [128, 128], BF, kind="ExternalInput").ap()
    d_idf = nc.dram_tensor("idf32", [128, 128], F32, kind="ExternalInput").ap()
    d_v3rep = nc.dram_tensor("v3rep", [128, 256], BF, kind="ExternalInput").ap()
    d_onesrow = nc.dram_tensor("onesrow", [1, 128], BF, kind="ExternalInput").ap()
    d_biasrow = nc.dram_tensor("biasrow", [1, 256], BF, kind="ExternalInput").ap()
    d_out = nc.dram_tensor("out", [U, S, H], F32, kind="ExternalOutput").ap()
    d_lbd = [
        nc.dram_tensor(f"lbd{u}", [S, 256], BF).ap()
        for u in range(U)
    ]

    with tile.TileContext(nc) as tc:
        with (
            tc.tile_pool(name="const", bufs=1) as cpool,
            tc.tile_pool(name="work", bufs=3) as wpool,
            tc.tile_pool(name="sumt", bufs=3) as sumpool,
            tc.tile_pool(name="tanh", bufs=3) as tpool,
            tc.tile_pool(name="pers", bufs=1) as ppool,
            tc.tile_pool(name="gate", bufs=3) as gpool,
            tc.tile_pool(name="psA", bufs=2, space=bass.MemorySpace.PSUM) as psA,
            tc.tile_pool(name="psL", bufs=1, space=bass.MemorySpace.PSUM) as psL,
            tc.tile_pool(name="psR", bufs=2, space=bass.MemorySpace.PSUM) as psR,
        ):
            pools = (cpool, wpool, sumpool, tpool, ppool, gpool, psA, psL, psR)

            def load(dram_ap, shape, dtype, tag):
                t = cpool.tile(shape, dtype, tag=tag)
                nc.sync.dma_start(t[:], dram_ap)
                return t

            sb = {
                "x": [load(d_xbf[u], [S, D], BF, f"x{u}") for u in range(U)],
                "xt": [
                    [
                        load(
                            d_xt[u, 128 * a : 128 * (a + 1), :],
                            [128, S], BF, f"xt{u}{a}",
                        )
                        for a in range(2)
                    ]
                    for u in range(U)
                ],
                "w1": [
                    load(d_w1[128 * a : 128 * (a + 1), :], [128, D], BF, f"w1{a}")
                    for a in range(2)
                ],
                "w2": [
                    load(d_w2[128 * a : 128 * (a + 1), :], [128, D], BF, f"w2{a}")
                    for a in range(2)
                ],
                "biasv": load(d_biasv, [128, 2], F32, "biasv"),
                "v3": load(d_v3, [128, 2], BF, "v3"),
                "gk": [
                    load(d_gk[128 * k : 128 * (k + 1), :], [128, 3 * H], BF, f"gk{k}")
                    for k in range(4)
                ],
                "rk": [
                    load(d_rk[128 * k : 128 * (k + 1), :], [128, 3 * H], BF, f"rk{k}")
                    for k in range(2)
                ],
                "btot": load(d_btot, [128, 8], F32, "btot"),
                "ones": load(d_ones, [128, 1], BF, "ones"),
                "idbf": load(d_idbf, [128, 128], BF, "idbf"),
                "idf": load(d_idf, [128, 128], F32, "idf"),
                "v3rep": load(d_v3rep, [128, 256], BF, "v3rep"),
                "onesrow": load(d_onesrow, [1, 128], BF, "onesrow"),
                "biasrow": load(d_biasrow, [1, 256], BF, "biasrow"),
                "lbd": d_lbd,
            }

            houtT = None
            for rep in range(reps):
                houtT = _body(nc, pools, sb, rep)
            for u in range(U):
                nc.sync.dma_start(d_out[u], houtT[:, u, :])

    return nc


@functools.lru_cache(maxsize=2)
def _graph(reps=1):
    return _build_graph(reps)


def _host_prep(features, w1, w2, bias, v3, gru_kernel, gru_rec_kernel, gru_bias):
    x = np.ascontiguousarray(np.asarray(features, np.float32).reshape(B * T, S, D))
    gb = np.asarray(gru_bias, np.float32)
    btot_main = (
        np.concatenate([(gb[0] + gb[1])[: 2 * H], gb[0][2 * H :]]).reshape(6, 128).T
    )
    btot = np.concatenate(
        [btot_main, gb[1][2 * H :].reshape(2, 128).T], axis=1
    ).astype(np.float32)
    params = {
        "w1bf": np.ascontiguousarray(np.asarray(w1).astype(BF_NP)),
        "w2bf": np.ascontiguousarray(np.asarray(w2).astype(BF_NP)),
        "biasv": np.ascontiguousarray(
            np.asarray(bias, np.float32).reshape(2, 128).T
        ),
        "v3bf": np.ascontiguousarray(
            np.asarray(v3, np.float32).reshape(2, 128).T.astype(BF_NP)
        ),
        "gkbf": np.ascontiguousarray(np.asarray(gru_kernel).astype(BF_NP)),
        "rkbf": np.ascontiguousarray(np.asarray(gru_rec_kernel).astype(BF_NP)),
        "btot": np.ascontiguousarray(btot),
        "onesbf": np.ones((128, 1), dtype=BF_NP),
        "v3rep": np.ascontiguousarray(
            np.broadcast_to(np.asarray(v3, np.float32)[None, :], (128, 256))
        ).astype(BF_NP),
        "onesrow": np.ones((1, 128), dtype=BF_NP),
        "biasrow": np.ascontiguousarray(
            np.asarray(bias, np.float32)[None, :]
        ).astype(BF_NP),
        "idbf": np.eye(128, dtype=np.float32).astype(BF_NP),
        "idf32": np.eye(128, dtype=np.float32),
    }
    in_maps = []
    for i in range(NCORES):
        xs = x[U * i : U * (i + 1)]
        m = dict(params)
        m["xbf"] = np.ascontiguousarray(xs.astype(BF_NP))
        m["xtbf"] = np.ascontiguousarray(xs.transpose(0, 2, 1).astype(BF_NP))
        in_maps.append(m)
    return in_maps


class _Runner:
    """Persistent jitted PJRT executor (mirrors bass2jax.run_bass_via_pjrt's
    multi-core path, but reuses the compiled executable across calls)."""

    def __init__(self, nc):
        import jax
        from jax.experimental.shard_map import shard_map
        from jax.sharding import Mesh, PartitionSpec

        from concourse import bass2jax as b2j
        from concourse import mybir as _mb

        b2j.install_neuronx_cc_hook()
        self.nc = nc
        in_names, out_names, out_avals, zero_outs = [], [], [], []
        partition_name = (
            nc.partition_id_tensor.name if nc.partition_id_tensor else None
        )
        for alloc in nc.m.functions[0].allocations:
            if not isinstance(alloc, _mb.MemoryLocationSet):
                continue
            name = alloc.memorylocations[0].name
            if alloc.kind == "ExternalInput":
                if name != partition_name:
                    in_names.append(name)
            elif alloc.kind == "ExternalOutput":
                shape = tuple(alloc.tensor_shape)
                dtype = _mb.dt.np(alloc.dtype)
                out_names.append(name)
                out_avals.append(jax.core.ShapedArray(shape, dtype))
                zero_outs.append(np.zeros(shape, dtype))
        self.in_names, self.out_names = in_names, out_names
        self.zero_outs = zero_outs
        n_params, n_outs = len(in_names), len(out_avals)
        all_names = tuple(
            in_names + out_names + ([partition_name] if partition_name else [])
        )

        def _bb(*args):
            operands = list(args)
            if partition_name is not None:
                operands.append(b2j.partition_id_tensor())
            return tuple(
                b2j._bass_exec_p.bind(
                    *operands,
                    out_avals=tuple(out_avals),
                    in_names=all_names,
                    out_names=tuple(out_names),
                    lowering_input_output_aliases=(),
                    sim_require_finite=True,
                    sim_require_nnan=True,
                    nc=nc,
                )
            )

        devices = jax.devices()[:NCORES]
        mesh = Mesh(np.asarray(devices), ("core",))
        in_specs = (PartitionSpec("core"),) * (n_params + n_outs)
        out_specs = (PartitionSpec("core"),) * n_outs
        self._fn = jax.jit(
            shard_map(
                _bb, mesh=mesh, in_specs=in_specs, out_specs=out_specs,
                check_rep=False,
            ),
            keep_unused=True,
        )

    def __call__(self, in_maps):
        concat_in = [
            np.concatenate([np.asarray(m[n]) for m in in_maps], axis=0)
            for n in self.in_names
        ]
        concat_zeros = [
            np.zeros((NCORES * z.shape[0], *z.shape[1:]), z.dtype)
            for z in self.zero_outs
        ]
        outs = [np.asarray(o) for o in self._fn(*concat_in, *concat_zeros)]
        res = []
        for c in range(NCORES):
            res.append(
                {
                    n: o[c * (o.shape[0] // NCORES) : (c + 1) * (o.shape[0] // NCORES)]
                    for n, o in zip(self.out_names, outs)
                }
            )
        return res


@functools.lru_cache(maxsize=2)
def _runner(reps=1):
    return _Runner(_graph(reps))


def run(inputs, reps=1):
    in_maps = _host_prep(**inputs)
    results = _runner(reps)(in_maps)
    outs = [np.asarray(results[i]["out"]) for i in range(NCORES)]
    full = np.concatenate(outs, axis=0).reshape(B, T, S, H).astype(np.float32)
    return full, results


def kernel(**inputs) -> np.ndarray:
    out, _ = run(inputs)
    return out
